# revision 11
# baseline (speedup 1.0000x reference)
"""Masked dot-product attention on 8 Trainium2 NeuronCores.

Problem shapes (hardcoded): queries/keys/values [128, 1024, 64] f32,
valid_lens [8] int (per-batch key valid length; BH = 8 batches x 16 heads).

Sharding: core c handles heads {b*16 + 2c, b*16 + 2c + 1} for all batches b
(16 heads/core, every batch present on every core -> uniform work, one
compiled program serves all cores).

Architecture (v2 - engine-balanced exp + q-tiled PV):
  - All matmul operands fp16 (1 cycle/row on PE, half DMA bytes, ~1e-4 dtype
    error). Scores S^T[k, q] computed per 128-key chunk into PSUM f32.
  - exp is the bottleneck op (only ScalarE has an exp table), so the softmax
    numerator columns are SPLIT across engines:
      * ACT: exp(s*0.125 + maskbias) -> fp16 P for q-columns [0:ACOLS)
      * DVE: Schraudolph fast-exp for q-columns [ACOLS:1024): one
        tensor_scalar (s * A16') + B16' with int16 output; the int16 bit
        pattern IS the fp16 approximation of exp (max rel err ~4%, rms ~1.8%).
        Per-partition scalar APs zero BOTH coefficients for masked keys so
        masked entries produce +0.0 exactly (no saturation dependence).
    Both write into ONE fp16 P tile (the DVE op through an int16 bitcast).
  - PV runs in the q-tiled direction: out[q,d] = P^T-tile.T @ V-chunk
    (lhsT = pt[:, c, t*128:(t+1)*128], rhs = v[:, c, 0:65] with a ones
    column accumulating the softmax denominator). Output lands directly as
    [q-partition, d] in PSUM - no PSUM->SBUF copies, no PE transposes, and
    normalization is a natural per-partition broadcast multiply.
  - Finalize per head: DVE reciprocal of the denominator column + two
    broadcast multiplies into an fp16 fin tile, single DMA out.
  - q-column permutation sigma: PV tile t, lhsT-free-index i holds query
    q = (t//4)*512 + 4*i + (t%4), so the fin tile [p, a, j, d] maps to
    DRAM rows a*512 + p*4 + j with contiguous 4*64*2B = 512B runs (full
    DMA bandwidth); sigma is applied host-side to Q^T columns only.
  - PSUM budget (8 banks): scores [128,1024] f32 x3 bufs (6) + two PV
    accumulators [128,4,65] f32 x1 buf (2).
  - PE FIFO order: per chunk slot, S-matmuls of chunk c then PV of chunk
    c-2 (two-slot lookahead keeps PE from waiting on exp).

Host side: fp16 conversion + layout packing (Q^T|K^T in one dram tensor),
V+ones chunk-major, mask bias tables, fp16->f32 output conversion, and the
reference's uniform-softmax patch for fully-masked batches (valid_len == 0).
"""

import numpy as np

P = 128          # partitions / k-chunk size
D = 64           # head dim
QL = 1024        # query length
KL = 1024        # key length
NB = 8           # batches
NH = 16          # heads per batch
NCORES = 8
HPC = 16         # heads per core
NCHUNK = KL // P # 8 k-chunks
NEG = -1.0e6

ACOLS = 656      # q-columns exp'd on ACT; rest via DVE Schraudolph
A16 = float(2**10 / np.log(2))   # fp16 Schraudolph slope (pre-scale)
C16 = 60.0                        # rms-optimal magic offset
B16 = float(15 * 1024 - C16)

_WARMUP = 4
_PV_LOOKAHEAD = 2


def _split_excess_waits(nc, max_waits=1):
    """This walrus (gen3) accepts only one sync-wait per instruction, but Tile
    emits up to 2 on compute ops and 5+ on the kernel-tail drain. Hoist excess
    on_wait entries onto fresh InstEventSemaphore ops on the same engine,
    inserted immediately before the offending instruction (same semantics:
    the engine stalls on each wait sequentially)."""
    import bass_rust
    import concourse.mybir as mybir

    n_split = 0
    for func in nc.m.functions:
        for block in func.blocks:
            out = []
            changed = False
            for inst in block.instructions:
                si = getattr(inst, "sync_info", None)
                waits = list(si.on_wait) if si is not None else []
                if len(waits) > max_waits:
                    changed = True
                    for w in waits[:-max_waits]:
                        n_split += 1
                        out.append(
                            mybir.InstEventSemaphore(
                                name=f"waitsplit_{n_split}_{inst.name}",
                                engine=inst.engine,
                                ins=[],
                                outs=[],
                                sync_info=bass_rust.SyncInfo(
                                    on_wait=[w], on_update=[]
                                ),
                            )
                        )
                    inst.sync_info = bass_rust.SyncInfo(
                        on_wait=waits[-max_waits:], on_update=list(si.on_update)
                    )
                out.append(inst)
            if changed:
                block.instructions = out
    return n_split


def _build(nc_chunks=None):
    import concourse.bass as bass
    import concourse.mybir as mybir
    from concourse.tile import TileContext

    if nc_chunks is None:
        nc_chunks = [NCHUNK] * NB

    f32 = mybir.dt.float32
    fp16 = mybir.dt.float16
    i16 = mybir.dt.int16
    Exp = mybir.ActivationFunctionType.Exp
    Alu = mybir.AluOpType

    nc = bass.Bass(trn_type="TRN2")
    qkd = nc.dram_tensor("qk", [HPC, D, QL + KL], fp16, kind="ExternalInput")
    vd = nc.dram_tensor("v", [HPC, P, NCHUNK, D + 1], fp16, kind="ExternalInput")
    md = nc.dram_tensor("mask", [P, NB * NCHUNK], f32, kind="ExternalInput")
    m2a = nc.dram_tensor("mask2a", [P, NB * NCHUNK], f32, kind="ExternalInput")
    m2b = nc.dram_tensor("mask2b", [P, NB * NCHUNK], f32, kind="ExternalInput")
    od = nc.dram_tensor("out", [HPC, QL, D], fp16, kind="ExternalOutput")

    with TileContext(nc) as tc:
        with (
            tc.tile_pool(name="consts", bufs=1) as consts,
            tc.tile_pool(name="io", bufs=3) as io,
            tc.tile_pool(name="pt", bufs=2) as ptp,
            tc.tile_pool(name="rc", bufs=4) as rcp,
            tc.tile_pool(name="fin", bufs=3) as finp,
            tc.tile_pool(name="ps_s", bufs=3, space="PSUM") as ps_s,
            tc.tile_pool(name="ps_oa", bufs=1, space="PSUM") as ps_oa,
            tc.tile_pool(name="ps_ob", bufs=1, space="PSUM") as ps_ob,
        ):
            # mask loads go on the GPSIMD SWDGE queue (parallel with the
            # HWDGE rings feeding the first matmuls)
            mask_sb = consts.tile([P, NB, NCHUNK], f32)
            nc.gpsimd.dma_start(
                out=mask_sb, in_=md.rearrange("p (b c) -> p b c", b=NB)
            )
            m2a_sb = consts.tile([P, NB, NCHUNK], f32)
            nc.gpsimd.dma_start(
                out=m2a_sb, in_=m2a.rearrange("p (b c) -> p b c", b=NB)
            )
            m2b_sb = consts.tile([P, NB, NCHUNK], f32)
            nc.gpsimd.dma_start(
                out=m2b_sb, in_=m2b.rearrange("p (b c) -> p b c", b=NB)
            )
            # prime the ScalarE exp table load (~1.3us) so it overlaps the
            # first input DMAs instead of stalling the first real exp
            scratch = consts.tile([1, 1], f32)
            nc.vector.memset(scratch, 0.0)
            nc.scalar.activation(scratch, scratch, Exp)
            # prime the PE clock ramp with short dummy matmuls; the output
            # shares the first head's PV accumulator slot (its c==0 matmul
            # start=True overwrite makes the garbage harmless)
            warm = consts.tile([1, D], fp16)
            nc.vector.memset(warm, 0.0)
            warm_ps = ps_oa.tile([P, 4, D + 1], f32, tag="oa")
            for _ in range(_WARMUP):
                nc.tensor.matmul(
                    warm_ps[0:1, 0, 0:D], warm[:, 0:1], warm,
                    start=True, stop=True,
                )

            def emit_front(h, first=False):
                b = h // 2
                nck = nc_chunks[b]
                qk = io.tile([D, QL + KL], fp16, tag="qk")
                if first:
                    # first exp needs qt + kt chunk 0 + mask. Three different
                    # sequencers so nothing serializes.
                    nc.scalar.dma_start(out=qk[:, 0:QL], in_=qkd[h][:, 0:QL])
                    nc.sync.dma_start(
                        out=qk[:, QL : QL + P], in_=qkd[h][:, QL : QL + P]
                    )
                    if nck > 1:
                        nc.sync.dma_start(
                            out=qk[:, QL + P : QL + nck * P],
                            in_=qkd[h][:, QL + P : QL + nck * P],
                        )
                else:
                    nc.sync.dma_start(
                        out=qk[:, 0 : QL + nck * P],
                        in_=qkd[h][:, 0 : QL + nck * P],
                    )
                v1 = io.tile([P, NCHUNK, D + 1], fp16, tag="v")
                nc.sync.dma_start(
                    out=v1[:, 0:nck, :], in_=vd[h][:, 0:nck, :]
                )
                pt = ptp.tile([P, NCHUNK, QL], fp16, tag="pt")
                return qk, v1, pt

            def emit_chunk(job):
                h, c, (qk, v1, pt) = job
                b = h // 2
                ps = ps_s.tile([P, QL], f32, tag="s")
                kt = qk[:, QL + c * P : QL + (c + 1) * P]
                # slab half first: its readers (DVE ts) free the PSUM buf
                # sooner, so the next S never queues behind a busy exp
                nc.tensor.matmul(
                    ps[:, 512:QL], kt, qk[:, 512:QL], start=True, stop=True
                )
                nc.tensor.matmul(
                    ps[:, 0:512], kt, qk[:, 0:512], start=True, stop=True
                )
                nc.scalar.activation(
                    pt[:, c, 0:ACOLS], ps[:, 0:ACOLS],
                    Exp, bias=mask_sb[:, b, c : c + 1], scale=0.125,
                )
                nc.vector.tensor_scalar(
                    pt[:, c, ACOLS:QL].bitcast(i16), ps[:, ACOLS:QL],
                    m2a_sb[:, b, c : c + 1], m2b_sb[:, b, c : c + 1],
                    Alu.mult, Alu.add,
                )

            def emit_pv(job, poqs, fin_ready):
                h, c, (qk, v1, pt) = job
                b = h // 2
                nck = nc_chunks[b]
                if c == 0:
                    poqs[h] = (
                        ps_oa.tile([P, 4, D + 1], f32, tag="oa", name="poq_a"),
                        ps_ob.tile([P, 4, D + 1], f32, tag="ob", name="poq_b"),
                    )
                poq_a, poq_b = poqs[h]
                for t in range(8):
                    dst = (poq_a if t < 4 else poq_b)[:, t % 4, :]
                    # start=True zeroes the full PSUM bank, so only the first
                    # of the four per-bank accumulation regions may use it;
                    # the others accumulate into the freshly-zeroed bank
                    nc.tensor.matmul(
                        dst,
                        pt[:, c, t * P : (t + 1) * P],
                        v1[:, c, :],
                        start=(c == 0 and t % 4 == 0), stop=(c == nck - 1),
                        skip_group_check=(t % 4 != 0),
                    )
                if c == nck - 1:
                    # don't emit the fin DVE ops yet: they wait on the PV
                    # stop-group, and sitting in DVE's in-order stream they
                    # would stall the next chunks' Schraudolph ops
                    fin_ready.append((h, poq_a, poq_b))

            def emit_fin(h, poq_a, poq_b):
                rc = rcp.tile([P, 2, 4], f32, tag="rc")
                nc.vector.reciprocal(rc[:, 0, :], poq_a[:, :, D : D + 1])
                nc.vector.reciprocal(rc[:, 1, :], poq_b[:, :, D : D + 1])
                fin = finp.tile([P, 2, 4, D], fp16, tag="fin")
                nc.vector.tensor_mul(
                    fin[:, 0], poq_a[:, :, 0:D],
                    rc[:, 0, :, None].broadcast_to([P, 4, D]),
                )
                nc.vector.tensor_mul(
                    fin[:, 1], poq_b[:, :, 0:D],
                    rc[:, 1, :, None].broadcast_to([P, 4, D]),
                )
                nc.sync.dma_start(
                    out=od[h].rearrange("(a p j) d -> p a j d", p=P, j=4),
                    in_=fin,
                )

            # flat chunk-job stream across heads with 2-slot PV lookahead;
            # prefetch head hi+1's inputs when head hi's first chunk starts.
            # Heads ordered by descending chunk count so the un-overlapped
            # pipeline tail belongs to the smallest head.
            order = [
                2 * bi + j
                for bi in sorted(range(NB), key=lambda b: -nc_chunks[b])
                for j in (0, 1)
            ]
            poqs = {}
            fin_ready = []
            front = {order[0]: emit_front(order[0], first=True)}
            plan = []  # (head_index_in_order, chunk)
            for hi, h in enumerate(order):
                for c in range(nc_chunks[h // 2]):
                    plan.append((hi, c))
            n_jobs = len(plan)
            jobs = [None] * n_jobs
            for i in range(n_jobs):
                hi, c = plan[i]
                h = order[hi]
                if c == 0 and hi + 1 < len(order):
                    front[order[hi + 1]] = emit_front(order[hi + 1])
                jobs[i] = (h, c, front[h])
                emit_chunk(jobs[i])
                while fin_ready:
                    emit_fin(*fin_ready.pop(0))
                if i >= _PV_LOOKAHEAD:
                    emit_pv(jobs[i - _PV_LOOKAHEAD], poqs, fin_ready)
            for i in range(max(0, n_jobs - _PV_LOOKAHEAD), n_jobs):
                emit_pv(jobs[i], poqs, fin_ready)
                while fin_ready:
                    emit_fin(*fin_ready.pop(0))
    _split_excess_waits(nc)
    return nc


_CACHE = {}


def _get_nc(key, nc_chunks):
    if key not in _CACHE:
        _CACHE[key] = _build(nc_chunks)
    return _CACHE[key]


def _core_head_idx(c):
    return [b * NH + 2 * c + j for b in range(NB) for j in range(2)]


def _sigma():
    # q index held by pt column t*128 + i  ->  (t//4)*512 + 4*i + (t%4)
    t = np.arange(NCHUNK)[:, None]
    i = np.arange(P)[None, :]
    return ((t // 4) * 512 + 4 * i + (t % 4)).reshape(-1)


def _prepare(queries, keys, values, valid_lens):
    queries = np.asarray(queries, np.float32)
    keys = np.asarray(keys, np.float32)
    values = np.asarray(values, np.float32)
    vl = np.asarray(valid_lens).astype(np.int64)
    key_pos = np.arange(KL)[None, :]
    maskbit = key_pos >= vl[:, None]             # [NB, KL] True=masked
    # ACT additive bias: [p, b, c] = 0 / -1e6 for key c*128+p
    mask = np.where(maskbit, np.float32(NEG), np.float32(0.0))
    mask_dev = np.ascontiguousarray(
        mask.reshape(NB, NCHUNK, P).transpose(2, 0, 1).reshape(P, NB * NCHUNK)
    ).astype(np.float32)
    # DVE Schraudolph coeffs, zeroed on masked keys (exact +0.0 output)
    m2a = np.where(maskbit, 0.0, A16 * 0.125).astype(np.float32)
    m2b = np.where(maskbit, 0.0, B16).astype(np.float32)
    m2a_dev = np.ascontiguousarray(
        m2a.reshape(NB, NCHUNK, P).transpose(2, 0, 1).reshape(P, NB * NCHUNK)
    )
    m2b_dev = np.ascontiguousarray(
        m2b.reshape(NB, NCHUNK, P).transpose(2, 0, 1).reshape(P, NB * NCHUNK)
    )
    nc_chunks = [max(1, int(min(NCHUNK, (int(v) + P - 1) // P))) for v in vl]
    bh = queries.shape[0]
    sig = _sigma()
    # packed [BH, 64, 2048]: Q^T sigma-permuted columns | K^T natural
    q16 = queries.astype(np.float16)
    k16 = keys.astype(np.float16)
    qk = np.empty((bh, D, QL + KL), np.float16)
    qk[:, :, 0:QL] = q16[:, sig, :].transpose(0, 2, 1)
    qk[:, :, QL:] = k16.transpose(0, 2, 1)
    # V chunk-major with ones column: [BH, 128, 8, 65]
    v1 = np.concatenate(
        [values, np.ones((bh, KL, 1), np.float32)], axis=-1
    ).astype(np.float16)
    v1p = np.ascontiguousarray(
        v1.reshape(bh, NCHUNK, P, D + 1).transpose(0, 2, 1, 3)
    )
    in_maps = []
    for c in range(NCORES):
        idx = _core_head_idx(c)
        in_maps.append(
            {
                "qk": qk[idx],
                "v": v1p[idx],
                "mask": mask_dev,
                "mask2a": m2a_dev,
                "mask2b": m2b_dev,
            }
        )
    return in_maps, nc_chunks, vl


def _run(in_maps, nc, trace=False):
    from concourse.bass_utils import run_bass_kernel_spmd

    return run_bass_kernel_spmd(
        nc, in_maps, core_ids=list(range(NCORES)), trace=trace
    )


def _gather(results, values, vl):
    out = np.empty((NB * NH, QL, D), np.float32)
    for c in range(NCORES):
        out[_core_head_idx(c)] = np.asarray(results[c]["out"], np.float32)
    # fully-masked batches: reference softmax(-1e6 * ones) is uniform
    for b in range(NB):
        if vl[b] == 0:
            for hh in range(NH):
                bh = b * NH + hh
                out[bh] = np.asarray(values[bh], np.float32).mean(
                    axis=0, keepdims=True
                )
    return out


def kernel(queries, keys, values, valid_lens):
    in_maps, nc_chunks, vl = _prepare(queries, keys, values, valid_lens)
    nc = _get_nc(tuple(nc_chunks), nc_chunks)
    res = _run(in_maps, nc)
    return _gather(res.results, values, vl)
